# revision 1
# baseline (speedup 1.0000x reference)
"""LiteSelfAttention2D on 8 trn2 NeuronCores.

Sharding: 16 (batch, head) jobs -> 2 per core (core c: b=c//2, heads {2*(c%2), 2*(c%2)+1}).
Each core computes partial output  Wp_h0 @ attn_h0 + Wp_h1 @ attn_h1  [256, 4096] (fp32);
host sums core pairs and adds the residual x.

Per-core dataflow (all layouts chosen so no cross-partition moves are ever needed):
  xf [256,4096] -> 2 SBUF c-half tiles [128,4096]
  Q_h replicated 2x on partitions:  Qr_h [64,4096]   (strip a in partitions 32a..32a+31 = Q_h)
  K_h split along m:                Kr_h [64,2048]   (strip a = K_h[:, 2048a + m'])
  V^T (+ ones col for the softmax denominator): VT_h [128, 32*33] bf16, block j at cols 33j
  scores (transposed!)  S^T[m-block, n-chunk] = Kr-slice.T @ Qr-slice   (2-way row-tiled K=32 matmuls)
  P^T = exp(S^T / sqrt(32))  via ACT (scale folded in), PSUM[128,1024] -> SBUF bf16
  out'[n,d|den] += P^T-block.T @ VT-block  (K=128, M=33, N=512, bf16) accumulated over 32 m-blocks
  onorm = out'[0:32] * bcast(1/out'[32])  (DVE recip -> K=1 ones-matmul broadcast -> DVE mult)
  partial = sum_h WpT_h.T @ onorm_h  (K=32 accumulating matmuls) -> DMA to DRAM

No max-subtraction in softmax: scores are ~N(0, 0.33) after scaling, exp is safe in fp32.
"""

import os
import sys

sys.path.insert(0, "/opt/trn_rl_repo")

import numpy as np
from contextlib import ExitStack

import concourse.bass as bass
import concourse.tile as tile
from concourse import bacc, mybir
from concourse._compat import with_exitstack

F32 = mybir.dt.float32
F32R = mybir.dt.float32r
BF16 = mybir.dt.bfloat16

B, C, HH, WW = 4, 256, 64, 64
N = HH * WW              # 4096
HEADS, HEAD_DIM, KEY_CH = 4, 32, 128
NCORES = 8
SCALE = 1.0 / float(np.sqrt(HEAD_DIM))
NB = N // 128            # 32 m-blocks
NS = N // 512            # 8 n-chunks


@with_exitstack
def _attention_kernel(ctx: ExitStack, tc: "tile.TileContext", out_ap, x_ap, wq2_ap, wk_ap, wv_ap, wp_ap):
    nc = tc.nc

    sb = ctx.enter_context(tc.tile_pool(name="sb", bufs=1))
    sb_pt = ctx.enter_context(tc.tile_pool(name="pt", bufs=3))
    sb_out = ctx.enter_context(tc.tile_pool(name="sb_out", bufs=3))
    ps_sc = ctx.enter_context(tc.tile_pool(name="ps_sc", bufs=2, space="PSUM"))
    ps_av = ctx.enter_context(tc.tile_pool(name="ps_av", bufs=2, space="PSUM"))
    ps_pr = ctx.enter_context(tc.tile_pool(name="ps_pr", bufs=2, space="PSUM"))

    # ---- persistent SBUF tensors ----
    xf = [sb.tile([128, N], F32R, tag=f"xf{ch}", name=f"xf{ch}") for ch in range(2)]
    wq2 = [sb.tile([128, 128], F32R, tag=f"wq2{ch}", name=f"wq2{ch}") for ch in range(2)]
    wk = [sb.tile([128, 256], F32R, tag=f"wk{ch}", name=f"wk{ch}") for ch in range(2)]
    wv = [sb.tile([128, 64], F32R, tag=f"wv{ch}", name=f"wv{ch}") for ch in range(2)]
    wp = [sb.tile([32, 256], F32, tag=f"wp{h}", name=f"wp{h}") for h in range(2)]
    qr = [sb.tile([64, N], BF16, tag=f"qr{h}", name=f"qr{h}") for h in range(2)]
    kr = [sb.tile([64, N // 2], BF16, tag=f"kr{h}", name=f"kr{h}") for h in range(2)]
    vt = [sb.tile([128, NB * 33], BF16, tag=f"vt{h}", name=f"vt{h}") for h in range(2)]
    onorm = [sb.tile([32, N], F32, tag=f"onorm{h}", name=f"onorm{h}") for h in range(2)]
    ones1 = sb.tile([1, 32], F32, tag="ones1", name="ones1")
    nc.vector.memset(ones1[:], 1.0)

    # ---- input DMAs ----
    for ch in range(2):
        for half in range(2):
            nc.sync.dma_start(
                out=xf[ch][:, bass.ts(half, N // 2)],
                in_=x_ap[128 * ch : 128 * (ch + 1), bass.ts(half, N // 2)],
            )
        nc.sync.dma_start(out=wq2[ch][:], in_=wq2_ap[128 * ch : 128 * (ch + 1), :])
        nc.sync.dma_start(out=wk[ch][:], in_=wk_ap[128 * ch : 128 * (ch + 1), :])
        nc.sync.dma_start(out=wv[ch][:], in_=wv_ap[128 * ch : 128 * (ch + 1), :])
    for h in range(2):
        nc.sync.dma_start(out=wp[h][:], in_=wp_ap[32 * h : 32 * (h + 1), :])

    # ---- V^T projection (both heads at once): VT[j] = xf_block_j.T @ WvT ----
    for h in range(2):
        nc.vector.memset(vt[h][:], 1.0)  # ones columns survive at 33j+32
    for j in range(NB):
        pv = ps_pr.tile([128, 64], F32, tag="ps_pr", name="pv")
        for ch in range(2):
            nc.tensor.matmul(
                out=pv[:],
                lhsT=xf[ch][:, bass.ts(j, 128)],
                rhs=wv[ch][:],
                start=(ch == 0),
                stop=(ch == 1),
            )
        for h in range(2):
            nc.vector.tensor_copy(vt[h][:, 33 * j : 33 * j + 32], pv[:, bass.ts(h, 32)])

    for h in range(2):
        # ---- Q replicated-2x projection: Qr_h[32a+d, n] = Q_h[d, n] ----
        for s in range(NS):
            pq = ps_pr.tile([64, 512], F32, tag="ps_pr", name="pqk")
            for ch in range(2):
                nc.tensor.matmul(
                    out=pq[:],
                    lhsT=wq2[ch][:, bass.ts(h, 64)],
                    rhs=xf[ch][:, bass.ts(s, 512)],
                    start=(ch == 0),
                    stop=(ch == 1),
                )
            nc.vector.tensor_copy(qr[h][:, bass.ts(s, 512)], pq[:])

        # ---- K split projection: Kr_h[32a+d, m'] = K_h[d, 2048a+m'] ----
        for s in range(4):
            pk = ps_pr.tile([64, 512], F32, tag="ps_pr", name="pqk")
            first = True
            for v in range(2):
                for ch in range(2):
                    nc.tensor.matmul(
                        out=pk[:],
                        lhsT=wk[ch][:, 128 * h + 64 * v : 128 * h + 64 * (v + 1)],
                        rhs=xf[ch][:, 2048 * v + 512 * s : 2048 * v + 512 * (s + 1)],
                        start=first,
                        stop=(v == 1 and ch == 1),
                    )
                    first = False
            nc.vector.tensor_copy(kr[h][:, bass.ts(s, 512)], pk[:])

    # ---- attention (heads sequential to keep PSUM within 8 banks) ----
    for h in range(2):
        for s in range(NS):
            outp = ps_av.tile([33, 512], F32, tag="ps_av", name="outp")
            for gp in range(16):
                sc = ps_sc.tile([128, 1024], F32, tag="ps_sc", name="sc")
                for a in range(2):
                    nc.tensor.matmul(
                        out=sc[:, bass.ts(a, 512)],
                        lhsT=kr[h][32 * a : 32 * (a + 1), bass.ts(gp, 128)],
                        rhs=qr[h][32 * a : 32 * (a + 1), bass.ts(s, 512)],
                        start=True,
                        stop=True,
                    )
                pt = sb_pt.tile([128, 1024], BF16, tag="pt", name="pt")
                nc.scalar.activation(
                    out=pt[:], in_=sc[:], func=mybir.ActivationFunctionType.Exp, scale=SCALE
                )
                for a in range(2):
                    j = gp + 16 * a
                    nc.tensor.matmul(
                        out=outp[:],
                        lhsT=vt[h][:, 33 * j : 33 * (j + 1)],
                        rhs=pt[:, bass.ts(a, 512)],
                        start=(gp == 0 and a == 0),
                        stop=(gp == 15 and a == 1),
                    )
            num_sb = sb_out.tile([32, 512], F32, tag="num_sb", name="num_sb")
            nc.vector.tensor_copy(num_sb[:], outp[0:32, :])
            rcp = sb_out.tile([1, 512], F32, tag="rcp", name="rcp")
            nc.vector.reciprocal(out=rcp[:], in_=outp[32:33, :])
            bc = ps_pr.tile([32, 512], F32, tag="ps_pr", name="bc")
            nc.tensor.matmul(out=bc[:], lhsT=ones1[:], rhs=rcp[:], start=True, stop=True)
            nc.vector.tensor_tensor(
                out=onorm[h][:, bass.ts(s, 512)],
                in0=bc[:],
                in1=num_sb[:],
                op=mybir.AluOpType.mult,
            )

    # ---- output projection: partial = sum_h WpT_h.T @ onorm_h ----
    for mh in range(2):
        for s in range(NS):
            po = ps_pr.tile([128, 512], F32, tag="ps_pr", name="po")
            for h in range(2):
                nc.tensor.matmul(
                    out=po[:],
                    lhsT=wp[h][:, bass.ts(mh, 128)],
                    rhs=onorm[h][:, bass.ts(s, 512)],
                    start=(h == 0),
                    stop=(h == 1),
                )
            po_sb = sb_out.tile([128, 512], F32, tag="po_sb", name="po_sb")
            nc.vector.tensor_copy(po_sb[:], po[:])
            nc.sync.dma_start(
                out=out_ap[128 * mh : 128 * (mh + 1), bass.ts(s, 512)], in_=po_sb[:]
            )


_CACHE = {}


def _build():
    if "nc" in _CACHE:
        return _CACHE["nc"]
    nc = bacc.Bacc("TRN2", target_bir_lowering=False, debug=False, num_devices=NCORES)
    x_t = nc.dram_tensor("x", [C, N], F32R, kind="ExternalInput").ap()
    wq2_t = nc.dram_tensor("wq2", [C, 128], F32R, kind="ExternalInput").ap()
    wk_t = nc.dram_tensor("wk", [C, 256], F32R, kind="ExternalInput").ap()
    wv_t = nc.dram_tensor("wv", [C, 64], F32R, kind="ExternalInput").ap()
    wp_t = nc.dram_tensor("wp", [64, C], F32, kind="ExternalInput").ap()
    out_t = nc.dram_tensor("out", [C, N], F32, kind="ExternalOutput").ap()
    with tile.TileContext(nc) as tc:
        _attention_kernel(tc, out_t, x_t, wq2_t, wk_t, wv_t, wp_t)
    nc.compile()
    _CACHE["nc"] = nc
    return nc


def make_in_maps(x, Wq, Wk, Wv, Wp):
    """Per-core input dicts (host-side prep: slicing + tiny transposes)."""
    xf = np.ascontiguousarray(x.reshape(B, C, N).astype(np.float32))
    in_maps = []
    for c in range(NCORES):
        b = c // 2
        h0 = 2 * (c % 2)
        heads = (h0, h0 + 1)
        wq2 = np.concatenate(
            [
                np.concatenate([Wq[32 * h : 32 * (h + 1), :].T] * 2, axis=1)
                for h in heads
            ],
            axis=1,
        )  # [256, 128]
        wk_blocks = []
        for h in heads:
            wt = Wk[32 * h : 32 * (h + 1), :].T  # [256, 32]
            z = np.zeros_like(wt)
            wk_blocks += [wt, z, z, wt]  # variant0 [W|0], variant1 [0|W]
        wk = np.concatenate(wk_blocks, axis=1)  # [256, 256]
        wv = np.concatenate([Wv[32 * h : 32 * (h + 1), :].T for h in heads], axis=1)
        wp = np.concatenate([Wp[:, 32 * h : 32 * (h + 1)].T for h in heads], axis=0)
        in_maps.append(
            {
                "x": xf[b],
                "wq2": np.ascontiguousarray(wq2, np.float32),
                "wk": np.ascontiguousarray(wk, np.float32),
                "wv": np.ascontiguousarray(wv, np.float32),
                "wp": np.ascontiguousarray(wp, np.float32),
            }
        )
    return in_maps


def kernel(x, Wq, Wk, Wv, Wp):
    from concourse.bass_utils import run_bass_kernel_spmd

    nc = _build()
    in_maps = make_in_maps(x, Wq, Wk, Wv, Wp)
    res = run_bass_kernel_spmd(nc, in_maps, list(range(NCORES)))
    xf = np.asarray(x, np.float32).reshape(B, C, N)
    out = np.empty((B, C, N), np.float32)
    for b in range(B):
        out[b] = res.results[2 * b]["out"] + res.results[2 * b + 1]["out"] + xf[b]
    return out.reshape(B, C, HH, WW)



# revision 2
# speedup vs baseline: 5.3097x; 5.3097x over previous
"""LiteSelfAttention2D on 8 trn2 NeuronCores — transfer-optimized.

Measured reality on this setup: the axon tunnel moves ~50 MB/s with ~50 ms
fixed cost per dispatch, while the on-device attention math is ~0.3 ms.  The
kernel is therefore designed around minimizing host<->device bytes:

Sharding: core c = (batch b=c//2, query-column-half q=c%2).  Each core
receives ONLY its own x slice  xh = x[b][:, 2048*q : 2048*(q+1)]  in bf16
(1 MB, no duplication across cores).  On device, a pair-wise AllGather
([0,1],[2,3],[4,5],[6,7]) reconstructs the full x[b] (needed for K/V over
all 4096 key positions).  Each core computes ALL 4 heads for its 2048
queries, applies the output projection AND the residual on device, and
returns a complete [256, 2048] bf16 slab — no host-side reduction.

Per warm call: ~10 MB up (8 MB x + 2 MB replicated weights) + 8 MB down,
instead of the previous ~96 MB (duplicated f32 x + donated zero buffers +
f32 partial outputs).  The jit'd dispatch closure is built once and cached
(run_bass_kernel_spmd re-traces jax.jit on every call).

Per-core dataflow (layouts avoid all cross-partition moves):
  xq      2 ch-half SBUF tiles [128, 2048] bf16   (own queries, residual)
  xk      2 ch-half SBUF tiles [128, 4096] bf16   (gathered full x[b])
  Qr_h    [64, 2048] bf16: Q_h replicated 2x on partitions (strip a = Q_h)
  Kr_h    [64, 2048] bf16: K_h split along keys (strip a = K_h[:, 2048a+m'])
  VT_h    [128, 33*32] bf16: V^T blocks + ones column for the softmax denom
  S^T     [128 keys, 512 queries] matmuls, K=32 contraction, 2 strips/PSUM
  P^T     exp(S^T/sqrt(32)) via scalar ACT (scale folded), PSUM->SBUF bf16
  out'    += P^T-block.T @ VT-block (K=128, M=33) over 32 key blocks
  onorm   out'[0:32] * bcast(1/out'[32])  -> bf16
  out     sum_h WpT_h.T @ onorm_h + xq  (residual on device) -> bf16 DMA

No max-subtraction in softmax: scores ~N(0, 0.33) after scaling, exp is safe.
"""

import sys

sys.path.insert(0, "/opt/trn_rl_repo")

import numpy as np
import ml_dtypes
from contextlib import ExitStack

import concourse.bass as bass
import concourse.tile as tile
from concourse import bacc, mybir
from concourse._compat import with_exitstack

F32 = mybir.dt.float32
BF16 = mybir.dt.bfloat16
BF16NP = ml_dtypes.bfloat16

B, C, HH, WW = 4, 256, 64, 64
N = HH * WW              # 4096 key positions
NQ = N // 2              # 2048 queries per core
HEADS, HEAD_DIM = 4, 32
NCORES = 8
SCALE = 1.0 / float(np.sqrt(HEAD_DIM))
NB = N // 128            # 32 key blocks
NSQ = NQ // 512          # 4 query chunks


@with_exitstack
def _attention_kernel(ctx: ExitStack, tc: "tile.TileContext", out_ap, xh_ap, wqkv_ap, wpt_ap):
    nc = tc.nc

    sb = ctx.enter_context(tc.tile_pool(name="sb", bufs=1))
    sb_pt = ctx.enter_context(tc.tile_pool(name="pt", bufs=3))
    sb_out = ctx.enter_context(tc.tile_pool(name="sb_out", bufs=3))
    ps_sc = ctx.enter_context(tc.tile_pool(name="ps_sc", bufs=2, space="PSUM"))
    ps_av = ctx.enter_context(tc.tile_pool(name="ps_av", bufs=2, space="PSUM"))
    ps_pr = ctx.enter_context(tc.tile_pool(name="ps_pr", bufs=2, space="PSUM"))
    dram = ctx.enter_context(tc.tile_pool(name="dram", bufs=1, space="DRAM"))

    # ---- pair-wise AllGather of x over DRAM (collectives can't touch I/O tensors) ----
    xb = dram.tile([C, NQ], BF16, tag="xb", name="xb")
    xg = dram.tile([2 * C, NQ], BF16, tag="xg", name="xg")
    nc.gpsimd.dma_start(out=xb[:], in_=xh_ap[:, :])
    nc.gpsimd.collective_compute(
        "AllGather",
        mybir.AluOpType.bypass,
        replica_groups=[[2 * b, 2 * b + 1] for b in range(B)],
        ins=[xb.opt()],
        outs=[xg.opt()],
    )

    # ---- persistent SBUF tensors ----
    xq = [sb.tile([128, NQ], BF16, tag=f"xq{ch}", name=f"xq{ch}") for ch in range(2)]
    xk = [sb.tile([128, N], BF16, tag=f"xk{ch}", name=f"xk{ch}") for ch in range(2)]
    raw = [sb.tile([128, 384], BF16, tag=f"raw{ch}", name=f"raw{ch}") for ch in range(2)]
    wq2 = [sb.tile([128, 64 * HEADS], BF16, tag=f"wq2{ch}", name=f"wq2{ch}") for ch in range(2)]
    wkz = [sb.tile([128, 128 * HEADS], BF16, tag=f"wkz{ch}", name=f"wkz{ch}") for ch in range(2)]
    wp = [sb.tile([32, 256], BF16, tag=f"wp{h}", name=f"wp{h}") for h in range(HEADS)]
    qr = [sb.tile([64, NQ], BF16, tag=f"qr{h}", name=f"qr{h}") for h in range(HEADS)]
    kr = [sb.tile([64, N // 2], BF16, tag=f"kr{h}", name=f"kr{h}") for h in range(HEADS)]
    vt = [sb.tile([128, NB * 33], BF16, tag=f"vt{h}", name=f"vt{h}") for h in range(HEADS)]
    onorm = [sb.tile([32, NQ], BF16, tag=f"onorm{h}", name=f"onorm{h}") for h in range(HEADS)]
    ones1 = sb.tile([1, 32], F32, tag="ones1", name="ones1")
    nc.vector.memset(ones1[:], 1.0)

    # ---- input DMAs ----
    for ch in range(2):
        nc.sync.dma_start(out=xq[ch][:], in_=xh_ap[128 * ch : 128 * (ch + 1), :])
        nc.sync.dma_start(out=raw[ch][:], in_=wqkv_ap[128 * ch : 128 * (ch + 1), :])
    for h in range(HEADS):
        nc.sync.dma_start(out=wp[h][:], in_=wpt_ap[32 * h : 32 * (h + 1), :])
    # gathered x: rows 0..255 = x[:, 0:2048], rows 256..511 = x[:, 2048:4096]
    for ch in range(2):
        nc.sync.dma_start(out=xk[ch][:, 0:NQ], in_=xg[128 * ch : 128 * (ch + 1), :])
        nc.sync.dma_start(out=xk[ch][:, NQ:N], in_=xg[C + 128 * ch : C + 128 * (ch + 1), :])

    # ---- derive packed weight layouts on device ----
    # wq2: per head h, cols 64h..64h+63 = [WqT_h | WqT_h]  (Q replicated 2x)
    # wkz: per head h, 128 cols: [WkT_h | 0 | 0 | WkT_h]   (K split in 2 strips)
    for ch in range(2):
        nc.vector.memset(wkz[ch][:], 0.0)
        for h in range(HEADS):
            qsrc = raw[ch][:, 32 * h : 32 * (h + 1)]
            nc.vector.tensor_copy(wq2[ch][:, 64 * h : 64 * h + 32], qsrc)
            nc.vector.tensor_copy(wq2[ch][:, 64 * h + 32 : 64 * h + 64], qsrc)
            ksrc = raw[ch][:, 128 + 32 * h : 128 + 32 * (h + 1)]
            nc.vector.tensor_copy(wkz[ch][:, 128 * h : 128 * h + 32], ksrc)
            nc.vector.tensor_copy(wkz[ch][:, 128 * h + 96 : 128 * h + 128], ksrc)

    # ---- Q projection from own slice: Qr_h[32a+d, n] = Q_h[d, n] ----
    for h in range(HEADS):
        for s in range(NSQ):
            pq = ps_pr.tile([64, 512], F32, tag="ps_pr", name="pq")
            for ch in range(2):
                nc.tensor.matmul(
                    out=pq[:],
                    lhsT=wq2[ch][:, 64 * h : 64 * (h + 1)],
                    rhs=xq[ch][:, bass.ts(s, 512)],
                    start=(ch == 0),
                    stop=(ch == 1),
                )
            nc.vector.tensor_copy(qr[h][:, bass.ts(s, 512)], pq[:])

    # ---- K projection from gathered x: Kr_h[32a+d, m'] = K_h[d, 2048a+m'] ----
    for h in range(HEADS):
        for s in range(4):
            pk = ps_pr.tile([64, 512], F32, tag="ps_pr", name="pk")
            first = True
            for a in range(2):
                for ch in range(2):
                    nc.tensor.matmul(
                        out=pk[:],
                        lhsT=wkz[ch][:, 128 * h + 64 * a : 128 * h + 64 * (a + 1)],
                        rhs=xk[ch][:, 2048 * a + 512 * s : 2048 * a + 512 * (s + 1)],
                        start=first,
                        stop=(a == 1 and ch == 1),
                    )
                    first = False
            nc.vector.tensor_copy(kr[h][:, bass.ts(s, 512)], pk[:])

    # ---- V^T projection (all heads at once): VT[j] = xk_block_j.T @ WvT ----
    for h in range(HEADS):
        nc.vector.memset(vt[h][:], 1.0)  # ones columns survive at 33j+32
    for j in range(NB):
        pv = ps_pr.tile([128, 128], F32, tag="ps_pr", name="pv")
        for ch in range(2):
            nc.tensor.matmul(
                out=pv[:],
                lhsT=xk[ch][:, bass.ts(j, 128)],
                rhs=raw[ch][:, 256:384],
                start=(ch == 0),
                stop=(ch == 1),
            )
        for h in range(HEADS):
            nc.vector.tensor_copy(vt[h][:, 33 * j : 33 * j + 32], pv[:, bass.ts(h, 32)])

    # ---- attention (heads sequential to keep PSUM within 8 banks) ----
    for h in range(HEADS):
        for s in range(NSQ):
            outp = ps_av.tile([33, 512], F32, tag="ps_av", name="outp")
            for gp in range(16):
                sc = ps_sc.tile([128, 1024], F32, tag="ps_sc", name="sc")
                for a in range(2):
                    nc.tensor.matmul(
                        out=sc[:, bass.ts(a, 512)],
                        lhsT=kr[h][32 * a : 32 * (a + 1), bass.ts(gp, 128)],
                        rhs=qr[h][32 * a : 32 * (a + 1), bass.ts(s, 512)],
                        start=True,
                        stop=True,
                    )
                pt = sb_pt.tile([128, 1024], BF16, tag="pt", name="pt")
                nc.scalar.activation(
                    out=pt[:], in_=sc[:], func=mybir.ActivationFunctionType.Exp, scale=SCALE
                )
                for a in range(2):
                    j = gp + 16 * a
                    nc.tensor.matmul(
                        out=outp[:],
                        lhsT=vt[h][:, 33 * j : 33 * (j + 1)],
                        rhs=pt[:, bass.ts(a, 512)],
                        start=(gp == 0 and a == 0),
                        stop=(gp == 15 and a == 1),
                    )
            num_sb = sb_out.tile([32, 512], F32, tag="num_sb", name="num_sb")
            nc.vector.tensor_copy(num_sb[:], outp[0:32, :])
            rcp = sb_out.tile([1, 512], F32, tag="rcp", name="rcp")
            nc.vector.reciprocal(out=rcp[:], in_=outp[32:33, :])
            bc = ps_pr.tile([32, 512], F32, tag="ps_pr", name="bc")
            nc.tensor.matmul(out=bc[:], lhsT=ones1[:], rhs=rcp[:], start=True, stop=True)
            nc.vector.tensor_tensor(
                out=onorm[h][:, bass.ts(s, 512)],
                in0=bc[:],
                in1=num_sb[:],
                op=mybir.AluOpType.mult,
            )

    # ---- output projection + residual: out = sum_h WpT_h.T @ onorm_h + xq ----
    for mh in range(2):
        for s in range(NSQ):
            po = ps_pr.tile([128, 512], F32, tag="ps_pr", name="po")
            for h in range(HEADS):
                nc.tensor.matmul(
                    out=po[:],
                    lhsT=wp[h][:, bass.ts(mh, 128)],
                    rhs=onorm[h][:, bass.ts(s, 512)],
                    start=(h == 0),
                    stop=(h == HEADS - 1),
                )
            po_sb = sb_out.tile([128, 512], BF16, tag="po_sb", name="po_sb")
            nc.vector.tensor_tensor(
                out=po_sb[:],
                in0=po[:],
                in1=xq[mh][:, bass.ts(s, 512)],
                op=mybir.AluOpType.add,
            )
            nc.sync.dma_start(
                out=out_ap[128 * mh : 128 * (mh + 1), bass.ts(s, 512)], in_=po_sb[:]
            )


_CACHE = {}


def _build():
    if "nc" in _CACHE:
        return _CACHE["nc"]
    nc = bacc.Bacc("TRN2", target_bir_lowering=False, debug=False, num_devices=NCORES)
    xh_t = nc.dram_tensor("xh", [C, NQ], BF16, kind="ExternalInput").ap()
    wqkv_t = nc.dram_tensor("wqkv", [C, 384], BF16, kind="ExternalInput").ap()
    wpt_t = nc.dram_tensor("wpt", [128, C], BF16, kind="ExternalInput").ap()
    out_t = nc.dram_tensor("out", [C, NQ], BF16, kind="ExternalOutput").ap()
    with tile.TileContext(nc) as tc:
        _attention_kernel(tc, out_t, xh_t, wqkv_t, wpt_t)
    nc.compile()
    _CACHE["nc"] = nc
    return nc


def _get_runner():
    """Cached jit'd SPMD dispatch (run_bass_kernel_spmd re-jits every call)."""
    if "runner" in _CACHE:
        return _CACHE["runner"]
    import jax
    from jax.sharding import Mesh, PartitionSpec
    from jax.experimental.shard_map import shard_map
    from concourse.bass2jax import _bass_exec_p, install_neuronx_cc_hook, partition_id_tensor

    nc = _build()
    install_neuronx_cc_hook()

    partition_name = nc.partition_id_tensor.name if nc.partition_id_tensor else None
    in_names = []
    out_names = []
    out_avals = []
    for alloc in nc.m.functions[0].allocations:
        if not isinstance(alloc, mybir.MemoryLocationSet):
            continue
        name = alloc.memorylocations[0].name
        if alloc.kind == "ExternalInput":
            if name != partition_name:
                in_names.append(name)
        elif alloc.kind == "ExternalOutput":
            out_names.append(name)
            out_avals.append(
                jax.core.ShapedArray(tuple(alloc.tensor_shape), mybir.dt.np(alloc.dtype))
            )
    n_params = len(in_names)
    in_names_full = list(in_names) + ([partition_name] if partition_name else [])

    def _body(*args):
        operands = list(args)
        if partition_name is not None:
            operands.append(partition_id_tensor())
        # No donated zero output buffers: the kernel writes every output
        # element, so uninitialized PJRT result buffers are fine and we
        # skip shipping zeros over the tunnel.
        outs = _bass_exec_p.bind(
            *operands,
            out_avals=tuple(out_avals),
            in_names=tuple(in_names_full),
            out_names=tuple(out_names),
            lowering_input_output_aliases=(),
            sim_require_finite=True,
            sim_require_nnan=True,
            nc=nc,
        )
        return tuple(outs)

    devices = jax.devices()[:NCORES]
    assert len(devices) == NCORES, f"need {NCORES} devices, have {len(jax.devices())}"
    mesh = Mesh(np.asarray(devices), ("core",))
    sharded = jax.jit(
        shard_map(
            _body,
            mesh=mesh,
            in_specs=(PartitionSpec("core"),) * n_params,
            out_specs=(PartitionSpec("core"),) * len(out_names),
            check_rep=False,
        )
    )

    def run(*global_inputs):
        out_arrs = sharded(*global_inputs)
        return np.asarray(out_arrs[0])

    _CACHE["runner"] = run
    return run


def make_global_inputs(x, Wq, Wk, Wv, Wp):
    """Global sharded input arrays (axis 0 split 8-ways across cores)."""
    xf = np.asarray(x, np.float32).reshape(B, C, 2, NQ)
    # core c = (b=c//2, half=c%2) gets x[b][:, half] -> [8*256, 2048] bf16
    xh_g = np.ascontiguousarray(xf.transpose(0, 2, 1, 3).reshape(NCORES * C, NQ).astype(BF16NP))
    wqkv = np.concatenate(
        [np.asarray(Wq, np.float32).T, np.asarray(Wk, np.float32).T, np.asarray(Wv, np.float32).T],
        axis=1,
    ).astype(BF16NP)  # [256, 384]
    wqkv_g = np.ascontiguousarray(np.broadcast_to(wqkv, (NCORES, C, 384))).reshape(NCORES * C, 384)
    wpt = np.asarray(Wp, np.float32).T.astype(BF16NP)  # [128, 256]
    wpt_g = np.ascontiguousarray(np.broadcast_to(wpt, (NCORES, 128, C))).reshape(NCORES * 128, C)
    return xh_g, wqkv_g, wpt_g


def assemble_output(out_g):
    """[8*256, 2048] bf16 -> [B, C, H, W] f32 (residual already applied)."""
    out = out_g.reshape(B, 2, C, NQ).transpose(0, 2, 1, 3).astype(np.float32)
    return out.reshape(B, C, HH, WW)


def kernel(x, Wq, Wk, Wv, Wp):
    run = _get_runner()
    out_g = run(*make_global_inputs(x, Wq, Wk, Wv, Wp))
    return assemble_output(out_g)


# revision 3
# speedup vs baseline: 9.2626x; 1.7445x over previous
"""LiteSelfAttention2D on 8 trn2 NeuronCores — transfer-optimized.

Measured reality on this setup: the axon tunnel moves ~50 MB/s with ~50 ms
fixed cost per dispatch, while the on-device attention math is ~0.3 ms.  The
kernel is therefore designed around minimizing host<->device bytes:

Sharding: core c = (batch b=c//2, query-column-half q=c%2).  Each core
receives ONLY its own x slice  xh = x[b][:, 2048*q : 2048*(q+1)]  as fp8
(e4m3, 0.5 MB — no duplication across cores).  On device, a pair-wise
AllGather ([0,1],[2,3],[4,5],[6,7]) reconstructs the full x[b] (needed for
K/V over all 4096 key positions).  QKV weights ship as per-core 1/8 chunks
and are 8-wide AllGathered on device (0.2 MB total instead of 2 MB
replicated).  Each core computes ALL 4 heads for its 2048 queries and
returns the softmax-normalized per-head attention output `onorm`
[4*32, 2048] in bf16 (0.5 MB).  The cheap final 1x1 projection
(Wp @ onorm, ~1 GFLOP) and the residual add run on the host in f32 — this
both halves the down-bytes and makes the residual exact, which buys back
the accuracy spent on fp8 inputs.

Per warm call: ~4.5 MB up + 4 MB down instead of the original ~96 MB
(duplicated f32 x + donated zero buffers + f32 partial outputs).  The
jit'd dispatch closure is built once and cached (run_bass_kernel_spmd
re-traces jax.jit on every call).

Per-core dataflow (layouts avoid all cross-partition moves):
  xq      2 ch-half SBUF tiles [128, 2048] fp8 -> bf16  (own queries)
  xk      2 ch-half SBUF tiles [128, 4096] fp8 -> bf16  (gathered x[b])
  Qr_h    [64, 2048] bf16: Q_h replicated 2x on partitions (strip a = Q_h)
  Kr_h    [64, 2048] bf16: K_h split along keys (strip a = K_h[:, 2048a+m'])
  VT_h    [128, 33*32] bf16: V^T blocks + ones column for the softmax denom
  S^T     [128 keys, 512 queries] matmuls, K=32 contraction, 2 strips/PSUM
  P^T     exp(S^T/sqrt(32)) via scalar ACT (scale folded), PSUM->SBUF bf16
  out'    += P^T-block.T @ VT-block (K=128, M=33) over 32 key blocks
  onorm_h out'[0:32] * bcast(1/out'[32]) -> bf16 -> DMA rows 32h of `out`

No max-subtraction in softmax: scores ~N(0, 0.33) after scaling, exp is safe.
"""

import sys

sys.path.insert(0, "/opt/trn_rl_repo")

import numpy as np
import ml_dtypes
from contextlib import ExitStack

import concourse.bass as bass
import concourse.tile as tile
from concourse import bacc, mybir
from concourse._compat import with_exitstack

F32 = mybir.dt.float32
BF16 = mybir.dt.bfloat16
XDT = mybir.dt.float8e4          # wire dtype for x (flip to BF16 if accuracy demands)
XDT_NP = mybir.dt.np(XDT)
BF16NP = ml_dtypes.bfloat16

B, C, HH, WW = 4, 256, 64, 64
N = HH * WW              # 4096 key positions
NQ = N // 2              # 2048 queries per core
HEADS, HEAD_DIM = 4, 32
NCORES = 8
SCALE = 1.0 / float(np.sqrt(HEAD_DIM))
NB = N // 128            # 32 key blocks
NSQ = NQ // 512          # 4 query chunks


@with_exitstack
def _attention_kernel(ctx: ExitStack, tc: "tile.TileContext", out_ap, xh_ap, wqkv_ap):
    nc = tc.nc

    sb = ctx.enter_context(tc.tile_pool(name="sb", bufs=1))
    sb_pt = ctx.enter_context(tc.tile_pool(name="pt", bufs=3))
    sb_out = ctx.enter_context(tc.tile_pool(name="sb_out", bufs=3))
    ps_sc = ctx.enter_context(tc.tile_pool(name="ps_sc", bufs=2, space="PSUM"))
    ps_av = ctx.enter_context(tc.tile_pool(name="ps_av", bufs=2, space="PSUM"))
    ps_pr = ctx.enter_context(tc.tile_pool(name="ps_pr", bufs=2, space="PSUM"))
    dram = ctx.enter_context(tc.tile_pool(name="dram", bufs=1, space="DRAM"))

    # ---- on-device gathers (collectives can't touch I/O tensors -> bounce) ----
    xb = dram.tile([C, NQ], XDT, tag="xb", name="xb")
    xg = dram.tile([2 * C, NQ], XDT, tag="xg", name="xg")
    nc.gpsimd.dma_start(out=xb[:], in_=xh_ap[:, :])
    nc.gpsimd.collective_compute(
        "AllGather",
        mybir.AluOpType.bypass,
        replica_groups=[[2 * b, 2 * b + 1] for b in range(B)],
        ins=[xb.opt()],
        outs=[xg.opt()],
    )
    wb = dram.tile([C // NCORES, 384], BF16, tag="wb", name="wb")
    wg = dram.tile([C, 384], BF16, tag="wg", name="wg")
    nc.gpsimd.dma_start(out=wb[:], in_=wqkv_ap[:, :])
    nc.gpsimd.collective_compute(
        "AllGather",
        mybir.AluOpType.bypass,
        replica_groups=[list(range(NCORES))],
        ins=[wb.opt()],
        outs=[wg.opt()],
    )

    # ---- persistent SBUF tensors ----
    xq8 = [sb.tile([128, NQ], XDT, tag=f"xq8{ch}", name=f"xq8{ch}") for ch in range(2)]
    xk8 = [sb.tile([128, N], XDT, tag=f"xk8{ch}", name=f"xk8{ch}") for ch in range(2)]
    xq = [sb.tile([128, NQ], BF16, tag=f"xq{ch}", name=f"xq{ch}") for ch in range(2)]
    xk = [sb.tile([128, N], BF16, tag=f"xk{ch}", name=f"xk{ch}") for ch in range(2)]
    raw = [sb.tile([128, 384], BF16, tag=f"raw{ch}", name=f"raw{ch}") for ch in range(2)]
    wq2 = [sb.tile([128, 64 * HEADS], BF16, tag=f"wq2{ch}", name=f"wq2{ch}") for ch in range(2)]
    wkz = [sb.tile([128, 128 * HEADS], BF16, tag=f"wkz{ch}", name=f"wkz{ch}") for ch in range(2)]
    qr = [sb.tile([64, NQ], BF16, tag=f"qr{h}", name=f"qr{h}") for h in range(HEADS)]
    kr = [sb.tile([64, N // 2], BF16, tag=f"kr{h}", name=f"kr{h}") for h in range(HEADS)]
    vt = [sb.tile([128, NB * 33], BF16, tag=f"vt{h}", name=f"vt{h}") for h in range(HEADS)]
    onorm = [sb.tile([32, NQ], BF16, tag=f"onorm{h}", name=f"onorm{h}") for h in range(HEADS)]
    ones1 = sb.tile([1, 32], F32, tag="ones1", name="ones1")
    nc.vector.memset(ones1[:], 1.0)

    # ---- input DMAs + fp8 -> bf16 upconvert ----
    for ch in range(2):
        nc.sync.dma_start(out=xq8[ch][:], in_=xh_ap[128 * ch : 128 * (ch + 1), :])
        nc.vector.tensor_copy(xq[ch][:], xq8[ch][:])
        nc.sync.dma_start(out=raw[ch][:], in_=wg[128 * ch : 128 * (ch + 1), :])
    # gathered x: rows 0..255 = x[:, 0:2048], rows 256..511 = x[:, 2048:4096]
    for ch in range(2):
        nc.sync.dma_start(out=xk8[ch][:, 0:NQ], in_=xg[128 * ch : 128 * (ch + 1), :])
        nc.sync.dma_start(out=xk8[ch][:, NQ:N], in_=xg[C + 128 * ch : C + 128 * (ch + 1), :])
        nc.vector.tensor_copy(xk[ch][:], xk8[ch][:])

    # ---- derive packed weight layouts on device ----
    # wq2: per head h, cols 64h..64h+63 = [WqT_h | WqT_h]  (Q replicated 2x)
    # wkz: per head h, 128 cols: [WkT_h | 0 | 0 | WkT_h]   (K split in 2 strips)
    for ch in range(2):
        nc.vector.memset(wkz[ch][:], 0.0)
        for h in range(HEADS):
            qsrc = raw[ch][:, 32 * h : 32 * (h + 1)]
            nc.vector.tensor_copy(wq2[ch][:, 64 * h : 64 * h + 32], qsrc)
            nc.vector.tensor_copy(wq2[ch][:, 64 * h + 32 : 64 * h + 64], qsrc)
            ksrc = raw[ch][:, 128 + 32 * h : 128 + 32 * (h + 1)]
            nc.vector.tensor_copy(wkz[ch][:, 128 * h : 128 * h + 32], ksrc)
            nc.vector.tensor_copy(wkz[ch][:, 128 * h + 96 : 128 * h + 128], ksrc)

    # ---- Q projection from own slice: Qr_h[32a+d, n] = Q_h[d, n] ----
    for h in range(HEADS):
        for s in range(NSQ):
            pq = ps_pr.tile([64, 512], F32, tag="ps_pr", name="pq")
            for ch in range(2):
                nc.tensor.matmul(
                    out=pq[:],
                    lhsT=wq2[ch][:, 64 * h : 64 * (h + 1)],
                    rhs=xq[ch][:, bass.ts(s, 512)],
                    start=(ch == 0),
                    stop=(ch == 1),
                )
            nc.vector.tensor_copy(qr[h][:, bass.ts(s, 512)], pq[:])

    # ---- K projection from gathered x: Kr_h[32a+d, m'] = K_h[d, 2048a+m'] ----
    for h in range(HEADS):
        for s in range(4):
            pk = ps_pr.tile([64, 512], F32, tag="ps_pr", name="pk")
            first = True
            for a in range(2):
                for ch in range(2):
                    nc.tensor.matmul(
                        out=pk[:],
                        lhsT=wkz[ch][:, 128 * h + 64 * a : 128 * h + 64 * (a + 1)],
                        rhs=xk[ch][:, 2048 * a + 512 * s : 2048 * a + 512 * (s + 1)],
                        start=first,
                        stop=(a == 1 and ch == 1),
                    )
                    first = False
            nc.vector.tensor_copy(kr[h][:, bass.ts(s, 512)], pk[:])

    # ---- V^T projection (all heads at once): VT[j] = xk_block_j.T @ WvT ----
    for h in range(HEADS):
        nc.vector.memset(vt[h][:], 1.0)  # ones columns survive at 33j+32
    for j in range(NB):
        pv = ps_pr.tile([128, 128], F32, tag="ps_pr", name="pv")
        for ch in range(2):
            nc.tensor.matmul(
                out=pv[:],
                lhsT=xk[ch][:, bass.ts(j, 128)],
                rhs=raw[ch][:, 256:384],
                start=(ch == 0),
                stop=(ch == 1),
            )
        for h in range(HEADS):
            nc.vector.tensor_copy(vt[h][:, 33 * j : 33 * j + 32], pv[:, bass.ts(h, 32)])

    # ---- attention (heads sequential to keep PSUM within 8 banks) ----
    for h in range(HEADS):
        for s in range(NSQ):
            outp = ps_av.tile([33, 512], F32, tag="ps_av", name="outp")
            for gp in range(16):
                sc = ps_sc.tile([128, 1024], F32, tag="ps_sc", name="sc")
                for a in range(2):
                    nc.tensor.matmul(
                        out=sc[:, bass.ts(a, 512)],
                        lhsT=kr[h][32 * a : 32 * (a + 1), bass.ts(gp, 128)],
                        rhs=qr[h][32 * a : 32 * (a + 1), bass.ts(s, 512)],
                        start=True,
                        stop=True,
                    )
                pt = sb_pt.tile([128, 1024], BF16, tag="pt", name="pt")
                nc.scalar.activation(
                    out=pt[:], in_=sc[:], func=mybir.ActivationFunctionType.Exp, scale=SCALE
                )
                for a in range(2):
                    j = gp + 16 * a
                    nc.tensor.matmul(
                        out=outp[:],
                        lhsT=vt[h][:, 33 * j : 33 * (j + 1)],
                        rhs=pt[:, bass.ts(a, 512)],
                        start=(gp == 0 and a == 0),
                        stop=(gp == 15 and a == 1),
                    )
            num_sb = sb_out.tile([32, 512], F32, tag="num_sb", name="num_sb")
            nc.vector.tensor_copy(num_sb[:], outp[0:32, :])
            rcp = sb_out.tile([1, 512], F32, tag="rcp", name="rcp")
            nc.vector.reciprocal(out=rcp[:], in_=outp[32:33, :])
            bc = ps_pr.tile([32, 512], F32, tag="ps_pr", name="bc")
            nc.tensor.matmul(out=bc[:], lhsT=ones1[:], rhs=rcp[:], start=True, stop=True)
            nc.vector.tensor_tensor(
                out=onorm[h][:, bass.ts(s, 512)],
                in0=bc[:],
                in1=num_sb[:],
                op=mybir.AluOpType.mult,
            )
            nc.sync.dma_start(
                out=out_ap[32 * h : 32 * (h + 1), bass.ts(s, 512)],
                in_=onorm[h][:, bass.ts(s, 512)],
            )


_CACHE = {}


def _build():
    if "nc" in _CACHE:
        return _CACHE["nc"]
    nc = bacc.Bacc("TRN2", target_bir_lowering=False, debug=False, num_devices=NCORES)
    xh_t = nc.dram_tensor("xh", [C, NQ], XDT, kind="ExternalInput").ap()
    wqkv_t = nc.dram_tensor("wqkv", [C // NCORES, 384], BF16, kind="ExternalInput").ap()
    out_t = nc.dram_tensor("out", [128, NQ], BF16, kind="ExternalOutput").ap()
    with tile.TileContext(nc) as tc:
        _attention_kernel(tc, out_t, xh_t, wqkv_t)
    nc.compile()
    _CACHE["nc"] = nc
    return nc


def _get_runner():
    """Cached jit'd SPMD dispatch (run_bass_kernel_spmd re-jits every call)."""
    if "runner" in _CACHE:
        return _CACHE["runner"]
    import jax
    from jax.sharding import Mesh, PartitionSpec
    from jax.experimental.shard_map import shard_map
    from concourse.bass2jax import _bass_exec_p, install_neuronx_cc_hook, partition_id_tensor

    nc = _build()
    install_neuronx_cc_hook()

    partition_name = nc.partition_id_tensor.name if nc.partition_id_tensor else None
    in_names = []
    out_names = []
    out_avals = []
    for alloc in nc.m.functions[0].allocations:
        if not isinstance(alloc, mybir.MemoryLocationSet):
            continue
        name = alloc.memorylocations[0].name
        if alloc.kind == "ExternalInput":
            if name != partition_name:
                in_names.append(name)
        elif alloc.kind == "ExternalOutput":
            out_names.append(name)
            out_avals.append(
                jax.core.ShapedArray(tuple(alloc.tensor_shape), mybir.dt.np(alloc.dtype))
            )
    n_params = len(in_names)
    in_names_full = list(in_names) + ([partition_name] if partition_name else [])

    def _body(*args):
        operands = list(args)
        if partition_name is not None:
            operands.append(partition_id_tensor())
        # No donated zero output buffers: the kernel writes every output
        # element, so uninitialized PJRT result buffers are fine and we
        # skip shipping zeros over the tunnel.
        outs = _bass_exec_p.bind(
            *operands,
            out_avals=tuple(out_avals),
            in_names=tuple(in_names_full),
            out_names=tuple(out_names),
            lowering_input_output_aliases=(),
            sim_require_finite=True,
            sim_require_nnan=True,
            nc=nc,
        )
        return tuple(outs)

    devices = jax.devices()[:NCORES]
    assert len(devices) == NCORES, f"need {NCORES} devices, have {len(jax.devices())}"
    mesh = Mesh(np.asarray(devices), ("core",))
    sharded = jax.jit(
        shard_map(
            _body,
            mesh=mesh,
            in_specs=(PartitionSpec("core"),) * n_params,
            out_specs=(PartitionSpec("core"),) * len(out_names),
            check_rep=False,
        )
    )

    def run(*global_inputs):
        out_arrs = sharded(*global_inputs)
        return np.asarray(out_arrs[0])

    _CACHE["runner"] = run
    return run


def make_global_inputs(x, Wq, Wk, Wv, Wp):
    """Global sharded input arrays (axis 0 split 8-ways across cores)."""
    xf = np.asarray(x, np.float32).reshape(B, C, 2, NQ)
    # core c = (b=c//2, half=c%2) gets x[b][:, half] -> [8*256, 2048]
    xh_g = np.ascontiguousarray(
        xf.transpose(0, 2, 1, 3).reshape(NCORES * C, NQ).astype(XDT_NP)
    )
    # [256, 384] bf16 = 8 cores x 32-row chunks; AllGathered back on device
    wqkv_g = np.concatenate(
        [np.asarray(Wq, np.float32).T, np.asarray(Wk, np.float32).T, np.asarray(Wv, np.float32).T],
        axis=1,
    ).astype(BF16NP)
    return xh_g, wqkv_g


def assemble_output(out_g, x, Wp):
    """[8*128, 2048] bf16 onorm -> host Wp projection + f32 residual."""
    on = (
        out_g.reshape(B, 2, 128, NQ)
        .transpose(0, 2, 1, 3)
        .reshape(B, 128, N)
        .astype(np.float32)
    )
    out = np.matmul(np.asarray(Wp, np.float32)[None], on)  # [B, 256, 4096]
    out += np.asarray(x, np.float32).reshape(B, C, N)
    return out.reshape(B, C, HH, WW)


def kernel(x, Wq, Wk, Wv, Wp):
    run = _get_runner()
    out_g = run(*make_global_inputs(x, Wq, Wk, Wv, Wp))
    return assemble_output(out_g, x, Wp)
